# revision 27
# baseline (speedup 1.0000x reference)
"""AttentionPooling GNN kernel for 8 Trainium2 NeuronCores.

Strategy
--------
Graph-parallel sharding: 128 graphs -> 16 graphs per core; each core gets its
graphs' nodes and (re-grouped) edges.  Host does index preprocessing only
(edge permutation by destination node, degree counts, weight folding); all
FLOPs on the edge/node payloads run on device.

Device algorithm (per core):
 1. Edges arrive sorted by destination row, padded so each node's run occupies
    whole 4-edge slots inside a single 128-edge chunk.  A shared triangular
    stationary (TriU4, bf16) turns chunk matmuls into slot-granular prefix
    sums P4; the four 32-column col-groups of the PE are packed via
    tile_position.  P4 spills to DRAM.
 2. Per-node edge sums A[n] = P4[hi_n] - P4[lo_n] (both in the same chunk,
    fetched with dma_gather; cancellation-free since both prefixes share all
    rounded terms except the node's own edges).  meanA = A * inv_cnt.
 3. The whole linear chain (node proj + edge proj + v-proj + attention scores)
    is folded host-side into one [194, 260] matrix: [h | meanA | has_edge | 1]
    @ W_big -> [v (256) | scores (4)].
 4. w = exp(scores) (segment-max skipped: scores are O(few), exp is safe in
    fp32; softmax ratio is unchanged).  Pooling = matmul with the per-chunk
    one-hot graph-membership matrix accumulated in PSUM -> [16, 260] of
    segment sums of [w*v | w].
 5. pooled = U/denom; out = pooled @ out_w.T + out_b (fp32).
"""
import sys

sys.path.insert(0, "/opt/trn_rl_repo")

import numpy as np

NUM_HEADS = 4
G_TOTAL = 128
CORES = 8
GL = G_TOTAL // CORES       # graphs per core
P = 128                     # partitions
SLOT = 4                    # edges per slot
SPC = 32                    # slots per chunk (128 edges)
CPG = 32                    # chunks per group (4096 edges)
GROUP_E = P * CPG           # 4096 edges per group
KB_LIST = [8, 8, 8, 7, 7, 6, 5, 3]   # node chunks per gather batch
S_BATCH = len(KB_LIST)
KB0 = [0]
for _kb in KB_LIST:
    KB0.append(KB0[-1] + _kb)
NC_NODES = KB0[-1] * P               # 6656 padded nodes per core
DENSE_LAG = 3


# ----------------------------------------------------------------- host prep
def _pack_core(deg):
    """Pack node edge-runs (padded to 4-edge slots) into 128-edge chunks.

    Every chunk starts with one pad slot (so lo = start-1 stays in-chunk) and
    no run straddles a chunk; each 128-node block starts a fresh chunk.
    Returns (s0 slots [NL], chunks per 128-node block [NB]).
    """
    NL = len(deg)
    r = (deg + SLOT - 1) // SLOT
    s0 = np.zeros(NL, np.int64)
    nblocks = (NL + P - 1) // P
    blk_chunks = np.zeros(nblocks, np.int64)
    cur = 0  # global slot cursor
    for kb in range(nblocks):
        if cur % SPC:
            cur = (cur // SPC + 1) * SPC
        start_chunk = cur // SPC
        for n in range(kb * P, min((kb + 1) * P, NL)):
            rn = r[n]
            if rn == 0:
                s0[n] = -1
                continue
            in_c = cur % SPC
            if in_c == 0:
                cur += 1
                in_c = 1
            if in_c + rn > SPC:
                cur = (cur // SPC + 1) * SPC + 1
            s0[n] = cur
            cur += rn
        blk_chunks[kb] = (cur + SPC - 1) // SPC - start_chunk
        cur = (cur + SPC - 1) // SPC * SPC
    return s0, r, blk_chunks


def _prep(h, edge_index, edge_attr, batch):
    """Shard + pack. Returns per-core dict of host arrays + shared config."""
    N = h.shape[0]
    row = np.asarray(edge_index[0], np.int64)
    batch = np.asarray(batch, np.int64)
    # graph -> node range (batch is sorted)
    gstart = np.searchsorted(batch, np.arange(G_TOTAL + 1))
    order = np.argsort(row, kind="stable")
    row_s = row[order]

    cores = []
    max_bgroups = 0
    for c in range(CORES):
        n0, n1 = int(gstart[GL * c]), int(gstart[GL * (c + 1)])
        NL = n1 - n0
        assert NL <= NC_NODES, (NL, NC_NODES)
        e0, e1 = np.searchsorted(row_s, [n0, n1])
        eord = order[e0:e1]
        lrow = row_s[e0:e1] - n0
        deg = np.bincount(lrow, minlength=NL)
        s0, r, blk_chunks = _pack_core(deg)
        # per-batch groups needed
        nb = len(blk_chunks)
        need = np.zeros(S_BATCH, np.int64)
        for s in range(S_BATCH):
            ch = blk_chunks[KB0[s]:KB0[s + 1]].sum()
            need[s] = max(1, (ch + CPG - 1) // CPG)
        cores.append(dict(n0=n0, n1=n1, NL=NL, eord=eord, lrow=lrow, deg=deg,
                          s0=s0, r=r, blk_chunks=blk_chunks, need=need))
    B_S = [max(int(st["need"][s]) for st in cores) for s in range(S_BATCH)]
    G0 = [0]
    for b in B_S:
        G0.append(G0[-1] + b)
    NGROUPS = G0[-1]
    E_PAD = NGROUPS * GROUP_E
    TROWS_MAX = max(B_S) * CPG * SPC
    assert TROWS_MAX <= 32767, TROWS_MAX

    for c, st in enumerate(cores):
        deg, s0, r, blk_chunks = st["deg"], st["s0"], st["r"], st["blk_chunks"]
        NL = st["NL"]
        # re-map block-local chunks to global chunks with batch alignment
        nb = len(blk_chunks)
        kb_batch = np.zeros(nb, np.int64)
        for s in range(S_BATCH):
            kb_batch[KB0[s]:KB0[s + 1]] = s
        blk_chunk0 = np.zeros(nb + 1, np.int64)
        cur_chunk = 0
        for kb in range(nb):
            s = int(kb_batch[kb])
            if kb == KB0[s]:
                cur_chunk = G0[s] * CPG
            blk_chunk0[kb] = cur_chunk
            cur_chunk += blk_chunks[kb]
            assert cur_chunk <= G0[s + 1] * CPG
        blk_chunk0[nb:] = cur_chunk
        # global slot of each node's run start (s0 was block-sequential)
        # recompute shift: s0 stored with blocks packed back-to-back from 0;
        # block kb originally started at chunk sum(blk_chunks[:kb]) (aligned)
        orig_start = np.zeros(nb, np.int64)
        acc = 0
        for kb in range(nb):
            orig_start[kb] = acc
            acc += blk_chunks[kb]
        shift = (blk_chunk0[:nb] - orig_start) * SPC  # slots to add per block
        node_blk = np.arange(NL) // P
        s0g = np.where(s0 >= 0, s0 + shift[node_blk], -1)

        # edge stream positions (logical), then swizzle to partition-major
        # DRAM layout: row_new = g*4096 + p*CPG + c  for logical edge
        # (g*4096 + c*128 + p) — so each SBUF partition reads one contiguous
        # 2048-element range per group (cheap DMA descriptors).
        first_edge = np.concatenate([[0], np.cumsum(deg)])[:-1]
        epos_base = np.repeat(4 * s0g[deg > 0], deg[deg > 0])
        within = np.arange(len(st["lrow"])) - np.repeat(first_edge[deg > 0], deg[deg > 0])
        epos = epos_base + within
        assert epos.max(initial=-1) < E_PAD * 1, (epos.max(initial=-1), E_PAD)
        eg = epos // GROUP_E
        ec = (epos % GROUP_E) // P
        ep = epos % P
        epos = eg * GROUP_E + ep * CPG + ec

        # table rows for hi / lo slots (batch-local)
        g0_arr = np.asarray(G0[:-1])

        def slot_to_row(sl_g):
            ch = sl_g // SPC
            sl = sl_g % SPC
            g = ch // CPG
            batch = np.searchsorted(np.asarray(G0[1:]), g, side="right")
            m = (ch % CPG) // 8
            j = (ch % CPG) % 8
            gg = g - g0_arr[batch]
            return gg * 1024 + (32 * m + sl) * 8 + j

        hi = np.where(s0g >= 0, s0g + r - 1, 0)
        lo = np.where(s0g >= 0, s0g - 1, 0)
        hi_row = np.where(s0g >= 0, slot_to_row(hi), 0).astype(np.int64)
        lo_row = np.where(s0g >= 0, slot_to_row(lo), 0).astype(np.int64)
        # pad nodes to NC_NODES
        hi_row = np.pad(hi_row, (0, NC_NODES - NL))
        lo_row = np.pad(lo_row, (0, NC_NODES - NL))
        assert hi_row.max() < TROWS_MAX and lo_row.max() < TROWS_MAX

        st.update(epos=epos, hi_row=hi_row, lo_row=lo_row)
    cfg = dict(B_S=B_S, G0=G0, NGROUPS=NGROUPS, E_PAD=E_PAD)
    return cores, cfg


def _wrap_idx(a, npart_rep=8):
    """[M] -> [128, M//16] int16, F-wrapped 16-row block replicated 8x."""
    m = a.reshape(-1, 16).T.astype(np.int16)          # [16, M/16]
    return np.tile(m, (npart_rep, 1))


def _fold_weights(node_w, node_b, edge_w, edge_b, query, in_w, in_b, out_w, out_b):
    D = query.shape[-1]
    dh = D // NUM_HEADS
    wq, wk, wv = in_w[:D], in_w[D:2 * D], in_w[2 * D:]
    bq, bk, bv = in_b[:D], in_b[D:2 * D], in_b[2 * D:]
    q = (query[0] @ wq.T + bq).reshape(NUM_HEADS, dh)
    s_w = np.einsum("hj,hjd->dh", q, wk.reshape(NUM_HEADS, dh, D)) / np.sqrt(dh)
    s_b = np.einsum("hj,hj->h", q, bk.reshape(NUM_HEADS, dh)) / np.sqrt(dh)
    # x_aug = [h(128) | meanA(64) | has_edge | 1] -> h_proj mapping A1 [194, 256]
    A1 = np.concatenate([node_w.T, edge_w.T, edge_b[None, :], node_b[None, :]], axis=0)
    M2 = np.concatenate([wv.T, s_w], axis=1)          # [256, 260]
    Wbig = A1 @ M2                                     # [194, 260]
    Wbig[-1, :256] += bv
    Wbig[-1, 256:] += s_b
    return Wbig.astype(np.float32)


# ------------------------------------------------------- numpy device model
def _numpy_device_model(cores, cfg, h, edge_attr, batch, Wbig, out_w, out_b):
    """Bit-approximate emulation of the device program (bf16 rounding where
    the device rounds) — used to validate packing/indexing host-side."""
    import ml_dtypes
    bf = lambda x: x.astype(ml_dtypes.bfloat16).astype(np.float32)
    B_S, G0, E_PAD = cfg["B_S"], cfg["G0"], cfg["E_PAD"]
    outs = []
    for c, st in enumerate(cores):
        NL = st["NL"]
        stream = np.zeros((E_PAD, 64), np.float32)
        stream[st["epos"]] = edge_attr[st["eord"]]
        streamb = bf(stream)
        # P4 tables per batch
        tables = []
        for s in range(S_BATCH):
            tab = np.zeros((32767, 64), np.float32)
            for gg in range(B_S[s]):
                g = G0[s] + gg
                for ch in range(CPG):
                    cdat = streamb[g * GROUP_E + np.arange(P) * CPG + ch]
                    pre = np.add.reduceat(cdat, np.arange(0, P, SLOT), 0).cumsum(0)
                    m, j = ch // 8, ch % 8
                    sl = np.arange(SPC)
                    tab[gg * 1024 + (32 * m + sl) * 8 + j] = pre
            tables.append(tab)
        nodechunk_batch = np.searchsorted(np.asarray(KB0[1:]),
                                          np.arange(NC_NODES) // P, side="right")
        tab_all = np.stack(tables)
        phi = tab_all[nodechunk_batch, st["hi_row"]]
        plo = tab_all[nodechunk_batch, st["lo_row"]]
        deg = np.pad(st["deg"], (0, NC_NODES - NL)).astype(np.float32)
        inv = 1.0 / np.maximum(deg, 1.0)
        meanA = bf((phi - plo) * inv[:, None])
        hase = (deg > 0).astype(np.float32)
        hpad = np.zeros((NC_NODES, 128), np.float32)
        hpad[:NL] = h[st["n0"]:st["n1"]]
        xaug = np.concatenate([bf(hpad), meanA, bf(hase[:, None]),
                               np.ones((NC_NODES, 1), np.float32)], 1)
        vs = bf(xaug) @ bf(Wbig)
        v, sc = vs[:, :256], vs[:, 256:]
        w = np.exp(sc)
        bl = np.full(NC_NODES, -1, np.int64)
        bl[:NL] = batch[st["n0"]:st["n1"]] - GL * c
        onehot = (bl[:, None] == np.arange(GL)[None, :]).astype(np.float32)
        wv4 = np.concatenate([bf(w[:, :, None] * v.reshape(-1, 4, 64)).reshape(-1, 256),
                              bf(w)], 1)
        U = bf(onehot).T @ wv4
        den = np.maximum(U[:, 256:], 1e-30)
        pooled = U[:, :256].reshape(GL, 4, 64) / den[:, :, None]
        o = pooled.reshape(GL, 256) @ out_w.T + out_b
        outs.append(o)
    return np.concatenate(outs).reshape(G_TOTAL, 1, 256)


# ------------------------------------------------------------- bass program
def _build_program(cfg):
    import concourse.bacc as bacc
    import concourse.mybir as mybir
    import concourse.tile as tile

    F32 = mybir.dt.float32
    BF16 = mybir.dt.bfloat16
    I16 = mybir.dt.int16
    AF = mybir.ActivationFunctionType
    B_S, G0, NGROUPS, E_PAD = cfg["B_S"], cfg["G0"], cfg["NGROUPS"], cfg["E_PAD"]
    NKB = NC_NODES // P            # 52 node chunks

    nc = bacc.Bacc("TRN2", num_devices=CORES, num_swdge_queues=4)
    es_d = nc.dram_tensor("es", [E_PAD, 64], BF16, kind="ExternalInput")
    h_d = nc.dram_tensor("h", [P, NC_NODES // P, 128], BF16, kind="ExternalInput")
    meta_d = nc.dram_tensor("meta", [P, NC_NODES // P, 4], F32, kind="ExternalInput")
    hi_d = nc.dram_tensor("hi", [P, NC_NODES // 16], I16, kind="ExternalInput")
    lo_d = nc.dram_tensor("lo", [P, NC_NODES // 16], I16, kind="ExternalInput")
    tri_d = nc.dram_tensor("tri", [P, SPC], BF16, kind="ExternalInput")
    idtb_d = nc.dram_tensor("idtb", [P, P], mybir.dt.bfloat16, kind="ExternalInput")
    idtf_d = nc.dram_tensor("idtf", [P, P], F32, kind="ExternalInput")
    iota_d = nc.dram_tensor("iota", [P, GL], F32, kind="ExternalInput")
    wb1_d = nc.dram_tensor("wb1", [128, 260], mybir.dt.bfloat16, kind="ExternalInput")
    wb2_d = nc.dram_tensor("wb2", [66, 260], mybir.dt.bfloat16, kind="ExternalInput")
    owt_d = nc.dram_tensor("owt", [256, 256], F32, kind="ExternalInput")
    ob_d = nc.dram_tensor("ob", [GL, 256], F32, kind="ExternalInput")
    y_d = nc.dram_tensor("y", [GL, 256], F32, kind="ExternalOutput")
    p4t = [nc.dram_tensor(f"p4t{s}", [B_S[s] * 1024, 64], F32, kind="Internal")
           for s in range(S_BATCH)]

    with tile.TileContext(nc) as tc:
        with tc.tile_pool(name="const", bufs=1) as cp, \
             tc.tile_pool(name="sb", bufs=3) as sb, \
             tc.tile_pool(name="big", bufs=1) as bigp, \
             tc.tile_pool(name="ps", bufs=2, space="PSUM") as ps, \
             tc.tile_pool(name="pacc", bufs=1, space="PSUM") as pacc:

            trib = cp.tile([P, SPC], BF16, name="trib")
            nc.sync.dma_start(out=trib[:], in_=tri_d.ap()[:, :])
            idtb = cp.tile([P, P], BF16, name="idtb")
            nc.sync.dma_start(out=idtb[:], in_=idtb_d.ap()[:, :])
            idtf = cp.tile([P, P], F32, name="idtf")
            nc.sync.dma_start(out=idtf[:], in_=idtf_d.ap()[:, :])
            iot = cp.tile([P, GL], F32, name="iot")
            nc.sync.dma_start(out=iot[:], in_=iota_d.ap()[:, :])
            wb1 = cp.tile([128, 260], BF16, name="wb1")
            nc.sync.dma_start(out=wb1[:], in_=wb1_d.ap()[:, :])
            wb2 = cp.tile([66, 260], BF16, name="wb2")
            nc.sync.dma_start(out=wb2[:], in_=wb2_d.ap()[:, :])
            owt = cp.tile([P, 2, 256], F32, name="owt")
            nc.sync.dma_start(out=owt[:], in_=owt_d.ap()[:, :].rearrange("(i p) f -> p i f", p=P))
            obt = cp.tile([GL, 256], F32, name="obt")
            nc.sync.dma_start(out=obt[:], in_=ob_d.ap()[:, :])
            hi_t = cp.tile([P, NC_NODES // 16], I16, name="hi_t")
            nc.sync.dma_start(out=hi_t[:], in_=hi_d.ap()[:, :])
            lo_t = cp.tile([P, NC_NODES // 16], I16, name="lo_t")
            nc.sync.dma_start(out=lo_t[:], in_=lo_d.ap()[:, :])

            # h shipped pre-transposed: partition = feature, free = (chunk, node)
            hsb = bigp.tile([P, NKB, 128], BF16, name="hsb")
            msb = bigp.tile([P, NKB, 4], F32, name="msb")
            phi = bigp.tile([P, NKB, 64], F32, name="phi")
            plo = bigp.tile([P, NKB, 64], F32, name="plo")
            augTall = bigp.tile([P, NKB, P], BF16, name="augTall")
            memall = bigp.tile([P, NKB, GL], BF16, name="memall")

            pool_ps = pacc.tile([GL, 260], F32, name="pool_ps")

            def emit_batch_dense(s):
                k0, KBB = KB0[s], KB_LIST[s]
                am = sb.tile([P, KBB, 64], F32, name="am", tag="am", bufs=2)
                nc.vector.tensor_sub(out=am[:], in0=phi[:, k0:k0 + KBB, :],
                                     in1=plo[:, k0:k0 + KBB, :])
                aug = sb.tile([P, KBB, 66], BF16, name="aug", tag="aug", bufs=2)
                nc.vector.tensor_tensor(
                    out=aug[:, :, :64], in0=am[:],
                    in1=msb[:, k0:k0 + KBB, 0].broadcast_to([P, KBB, 64]),
                    op=mybir.AluOpType.mult)
                nc.vector.tensor_copy(out=aug[:, :, 64:66],
                                      in_=msb[:, k0:k0 + KBB, 1:3])
                nc.vector.tensor_tensor(
                    out=memall[:, k0:k0 + KBB, :],
                    in0=iot[:].broadcast_to([P, GL, KBB]).rearrange("p g k -> p k g"),
                    in1=msb[:, k0:k0 + KBB, 3].broadcast_to([P, KBB, GL]),
                    op=mybir.AluOpType.is_equal)
                for k in range(k0, k0 + KBB):
                    pta = ps.tile([66, P], BF16, name="pta", tag="pta", bufs=2)
                    nc.tensor.transpose(out=pta[:], in_=aug[:, k - k0, :66],
                                        identity=idtb[:])
                    nc.scalar.copy(out=augTall[:66, k, :], in_=pta[:])
                for k in range(k0, k0 + KBB):
                    vs = ps.tile([P, 260], F32, name="vs", tag="vs", bufs=2)
                    nc.tensor.matmul(out=vs[:], lhsT=hsb[:, k, :], rhs=wb1[:],
                                     start=True, stop=False)
                    nc.tensor.matmul(out=vs[:], lhsT=augTall[:66, k, :], rhs=wb2[:],
                                     start=False, stop=True)
                    wsb = sb.tile([P, 4], F32, name="wsb", tag="wsb", bufs=4)
                    nc.scalar.activation(out=wsb[:], in_=vs[:, 256:260], func=AF.Exp)
                    pr = sb.tile([P, 260], BF16, name="pr", tag="pr", bufs=4)
                    nc.vector.tensor_tensor(
                        out=pr[:, :256].rearrange("p (h f) -> p h f", h=NUM_HEADS),
                        in0=vs[:, :256].rearrange("p (h f) -> p h f", h=NUM_HEADS),
                        in1=wsb[:].broadcast_to([P, NUM_HEADS, 64]),
                        op=mybir.AluOpType.mult)
                    nc.vector.tensor_copy(out=pr[:, 256:260], in_=wsb[:])
                    nc.tensor.matmul(out=pool_ps[:], lhsT=memall[:, k, :], rhs=pr[:],
                                     start=(k == 0), stop=(k == NKB - 1))

            for g in range(NGROUPS):
                et = sb.tile([P, CPG, 64], BF16, name="et", tag="et", bufs=6)
                nc.sync.dma_start(
                    out=et[:],
                    in_=es_d.ap()[g * GROUP_E:(g + 1) * GROUP_E, :]
                        .rearrange("(p c) f -> p c f", p=P))
                pp = ps.tile([P, 512], F32, name="pp", tag="pp", bufs=3)
                for m in range(4):
                    nc.tensor.matmul(
                        out=pp[32 * m:32 * m + 32, :],
                        lhsT=trib[:],
                        rhs=et[:, 8 * m:8 * m + 8, :].rearrange("p c f -> p (c f)"),
                        start=True, stop=True,
                        tile_position=(0, 32 * m))
                p4sb = sb.tile([P, 512], F32, name="p4sb", tag="p4sb", bufs=6)
                nc.vector.tensor_copy(out=p4sb[:], in_=pp[:])
                import bisect
                s = bisect.bisect_right(G0, g) - 1
                gg = g - G0[s]
                # spill on the ACT HWDGE queue so edge loads (sync queue) flow
                nc.scalar.dma_start(
                    out=p4t[s].ap()[gg * 1024:(gg + 1) * 1024, :]
                        .rearrange("(q x) f -> q (x f)", q=P),
                    in_=p4sb[:])
                if gg != B_S[s] - 1:
                    continue
                if s == 0:
                    # big node loads on the scalar HWDGE queue: off the
                    # latency-critical sync queue that feeds edge tiles
                    nc.gpsimd.dma_start(out=hsb[:], in_=h_d.ap()[:, :, :])
                    nc.gpsimd.dma_start(out=msb[:], in_=meta_d.ap()[:, :, :])
                # ---- batch s fully spilled: issue gathers now; defer the
                # dense work one batch so gather latency hides behind the
                # next batch's prefix matmuls (PE queue is in-order).
                k0, KBB = KB0[s], KB_LIST[s]
                c0 = KB0[s] * 8
                halves = [(0, KBB // 2), (KBB // 2, KBB)]
                qn = 0
                for tgt, idxt_t in ((phi, hi_t), (plo, lo_t)):
                    for (a, b) in halves:
                        nidx = (b - a) * P
                        nc.gpsimd.dma_gather(
                            out_ap=tgt[:, k0 + a:k0 + b, :],
                            in_ap=p4t[s].ap()[:, :],
                            idxs_ap=idxt_t[:, c0 + a * 8:c0 + b * 8],
                            num_idxs=nidx, num_idxs_reg=nidx, elem_size=64,
                            single_packet=False, queue_num=qn)
                        qn = (qn + 1) % 4
                if s >= DENSE_LAG:
                    emit_batch_dense(s - DENSE_LAG)

            for s in range(max(0, S_BATCH - DENSE_LAG), S_BATCH):
                emit_batch_dense(s)

            # ---- final: normalize + output projection
            den = sb.tile([GL, 4], F32, name="den")
            nc.vector.tensor_scalar_max(out=den[:], in0=pool_ps[:, 256:260],
                                        scalar1=1e-30)
            rden = sb.tile([GL, 4], F32, name="rden")
            nc.vector.reciprocal(out=rden[:], in_=den[:])
            pn = sb.tile([GL, 256], F32, name="pn")
            for hh in range(NUM_HEADS):
                nc.vector.tensor_scalar_mul(out=pn[:, 64 * hh:64 * hh + 64],
                                            in0=pool_ps[:, 64 * hh:64 * hh + 64],
                                            scalar1=rden[:, hh:hh + 1])
            pnT = sb.tile([P, 2, GL], F32, name="pnT")
            for i in range(2):
                ptp = ps.tile([P, GL], F32, name="ptp", tag="pta", bufs=2)
                nc.tensor.transpose(out=ptp[:], in_=pn[:, i * P:(i + 1) * P],
                                    identity=idtf[:GL, :GL])
                nc.vector.tensor_copy(out=pnT[:, i, :], in_=ptp[:])
            ops_t = ps.tile([GL, 256], F32, name="ops_t", tag="pta", bufs=2)
            for i in range(2):
                nc.tensor.matmul(out=ops_t[:], lhsT=pnT[:, i, :], rhs=owt[:, i, :],
                                 start=(i == 0), stop=(i == 1))
            osb = sb.tile([GL, 256], F32, name="osb")
            nc.vector.tensor_add(out=osb[:], in0=ops_t[:], in1=obt[:])
            nc.sync.dma_start(out=y_d.ap()[:, :], in_=osb[:])

    nc.finalize()
    return nc


_CACHE = {}


def _get_program(cfg):
    key = tuple(cfg["B_S"])
    if key not in _CACHE:
        _CACHE[key] = _build_program(cfg)
    return _CACHE[key]


def kernel(h, edge_index, edge_attr, batch, num_graphs,
           node_w, node_b, edge_w, edge_b, query, in_w, in_b, out_w, out_b,
           _trace=False, _numpy_only=False):
    import ml_dtypes
    h = np.asarray(h, np.float32)
    edge_attr = np.asarray(edge_attr, np.float32)
    batch_np = np.asarray(batch, np.int64)
    assert int(num_graphs) == G_TOTAL

    cores, cfg = _prep(h, edge_index, edge_attr, batch_np)
    Wbig = _fold_weights(np.asarray(node_w, np.float32), np.asarray(node_b, np.float32),
                         np.asarray(edge_w, np.float32), np.asarray(edge_b, np.float32),
                         np.asarray(query, np.float32), np.asarray(in_w, np.float32),
                         np.asarray(in_b, np.float32), np.asarray(out_w, np.float32),
                         np.asarray(out_b, np.float32))
    if _numpy_only:
        return _numpy_device_model(cores, cfg, h, edge_attr, batch_np, Wbig,
                                   np.asarray(out_w, np.float32),
                                   np.asarray(out_b, np.float32))

    bf16 = ml_dtypes.bfloat16
    tri = (np.arange(P)[:, None] // SLOT <= np.arange(SPC)[None, :]).astype(np.float32)
    idt = np.eye(P)
    iota = np.tile(np.arange(GL, dtype=np.float32)[None, :], (P, 1))
    shared = dict(
        tri=tri.astype(bf16),
        idtb=idt.astype(bf16), idtf=idt.astype(np.float32),
        iota=iota,
        wb1=Wbig[:128].astype(bf16), wb2=Wbig[128:].astype(bf16),
        owt=np.ascontiguousarray(np.asarray(out_w, np.float32).T),
        ob=np.tile(np.asarray(out_b, np.float32)[None, :], (GL, 1)),
    )
    in_maps = []
    for c, st in enumerate(cores):
        NL = st["NL"]
        stream = np.zeros((cfg["E_PAD"], 64), np.float32)
        stream[st["epos"]] = edge_attr[st["eord"]]
        hpad = np.zeros((NC_NODES, 128), np.float32)
        hpad[:NL] = h[st["n0"]:st["n1"]]
        deg = np.pad(st["deg"], (0, NC_NODES - NL)).astype(np.float32)
        meta = np.stack([1.0 / np.maximum(deg, 1.0),
                         (deg > 0).astype(np.float32),
                         np.ones(NC_NODES, np.float32),
                         np.concatenate([batch_np[st["n0"]:st["n1"]] - GL * c,
                                         np.full(NC_NODES - NL, -1.0)]).astype(np.float32)],
                        axis=1).astype(np.float32)
        h3 = np.ascontiguousarray(hpad.reshape(-1, P, 128).transpose(2, 0, 1))
        meta3 = np.ascontiguousarray(meta.reshape(-1, P, 4).transpose(1, 0, 2))
        in_maps.append(dict(es=stream.astype(bf16), h=h3.astype(bf16), meta=meta3,
                            hi=_wrap_idx(st["hi_row"]), lo=_wrap_idx(st["lo_row"]),
                            **shared))

    from concourse.bass_utils import run_bass_kernel_spmd
    nc = _get_program(cfg)
    res = run_bass_kernel_spmd(nc, in_maps, core_ids=list(range(CORES)),
                               trace=_trace)
    out = np.concatenate([np.asarray(res.results[c]["y"], np.float32)
                          for c in range(CORES)])
    kernel._last_result = res
    return out.reshape(G_TOTAL, 1, 256)


# revision 28
# speedup vs baseline: 1.0584x; 1.0584x over previous
"""AttentionPooling GNN kernel for 8 Trainium2 NeuronCores.

Strategy
--------
Graph-parallel sharding: 128 graphs -> 16 graphs per core; each core gets its
graphs' nodes and (re-grouped) edges.  Host does index preprocessing only
(edge permutation by destination node, degree counts, weight folding); all
FLOPs on the edge/node payloads run on device.

Device algorithm (per core):
 1. Edges arrive sorted by destination row, padded so each node's run occupies
    whole 4-edge slots inside a single 128-edge chunk.  A shared triangular
    stationary (TriU4, bf16) turns chunk matmuls into slot-granular prefix
    sums P4; the four 32-column col-groups of the PE are packed via
    tile_position.  P4 spills to DRAM.
 2. Per-node edge sums A[n] = P4[hi_n] - P4[lo_n] (both in the same chunk,
    fetched with dma_gather; cancellation-free since both prefixes share all
    rounded terms except the node's own edges).  meanA = A * inv_cnt.
 3. The whole linear chain (node proj + edge proj + v-proj + attention scores)
    is folded host-side into one [194, 260] matrix: [h | meanA | has_edge | 1]
    @ W_big -> [v (256) | scores (4)].
 4. w = exp(scores) (segment-max skipped: scores are O(few), exp is safe in
    fp32; softmax ratio is unchanged).  Pooling = matmul with the per-chunk
    one-hot graph-membership matrix accumulated in PSUM -> [16, 260] of
    segment sums of [w*v | w].
 5. pooled = U/denom; out = pooled @ out_w.T + out_b (fp32).
"""
import sys

sys.path.insert(0, "/opt/trn_rl_repo")

import numpy as np

NUM_HEADS = 4
G_TOTAL = 128
CORES = 8
GL = G_TOTAL // CORES       # graphs per core
P = 128                     # partitions
SLOT = 4                    # edges per slot
SPC = 32                    # slots per chunk (128 edges)
CPG = 32                    # chunks per group (4096 edges)
GROUP_E = P * CPG           # 4096 edges per group
KB_LIST = [8, 8, 8, 7, 7, 6, 5, 3]   # node chunks per gather batch
S_BATCH = len(KB_LIST)
KB0 = [0]
for _kb in KB_LIST:
    KB0.append(KB0[-1] + _kb)
NC_NODES = KB0[-1] * P               # 6656 padded nodes per core
DENSE_LAG = 3


# ----------------------------------------------------------------- host prep
def _pack_core(deg):
    """Pack node edge-runs (padded to 4-edge slots) into 128-edge chunks.

    Every chunk starts with one pad slot (so lo = start-1 stays in-chunk) and
    no run straddles a chunk; each 128-node block starts a fresh chunk.
    Returns (s0 slots [NL], chunks per 128-node block [NB]).
    """
    NL = len(deg)
    r = (deg + SLOT - 1) // SLOT
    s0 = np.zeros(NL, np.int64)
    nblocks = (NL + P - 1) // P
    blk_chunks = np.zeros(nblocks, np.int64)
    cur = 0  # global slot cursor
    for kb in range(nblocks):
        if cur % SPC:
            cur = (cur // SPC + 1) * SPC
        start_chunk = cur // SPC
        for n in range(kb * P, min((kb + 1) * P, NL)):
            rn = r[n]
            if rn == 0:
                s0[n] = -1
                continue
            in_c = cur % SPC
            if in_c == 0:
                cur += 1
                in_c = 1
            if in_c + rn > SPC:
                cur = (cur // SPC + 1) * SPC + 1
            s0[n] = cur
            cur += rn
        blk_chunks[kb] = (cur + SPC - 1) // SPC - start_chunk
        cur = (cur + SPC - 1) // SPC * SPC
    return s0, r, blk_chunks


def _prep(h, edge_index, edge_attr, batch):
    """Shard + pack. Returns per-core dict of host arrays + shared config."""
    N = h.shape[0]
    row = np.asarray(edge_index[0], np.int64)
    batch = np.asarray(batch, np.int64)
    # graph -> node range (batch is sorted)
    gstart = np.searchsorted(batch, np.arange(G_TOTAL + 1))
    order = np.argsort(row, kind="stable")
    row_s = row[order]

    cores = []
    max_bgroups = 0
    for c in range(CORES):
        n0, n1 = int(gstart[GL * c]), int(gstart[GL * (c + 1)])
        NL = n1 - n0
        assert NL <= NC_NODES, (NL, NC_NODES)
        e0, e1 = np.searchsorted(row_s, [n0, n1])
        eord = order[e0:e1]
        lrow = row_s[e0:e1] - n0
        deg = np.bincount(lrow, minlength=NL)
        s0, r, blk_chunks = _pack_core(deg)
        # per-batch groups needed
        nb = len(blk_chunks)
        need = np.zeros(S_BATCH, np.int64)
        for s in range(S_BATCH):
            ch = blk_chunks[KB0[s]:KB0[s + 1]].sum()
            need[s] = max(1, (ch + CPG - 1) // CPG)
        cores.append(dict(n0=n0, n1=n1, NL=NL, eord=eord, lrow=lrow, deg=deg,
                          s0=s0, r=r, blk_chunks=blk_chunks, need=need))
    B_S = [max(int(st["need"][s]) for st in cores) for s in range(S_BATCH)]
    G0 = [0]
    for b in B_S:
        G0.append(G0[-1] + b)
    NGROUPS = G0[-1]
    E_PAD = NGROUPS * GROUP_E
    TROWS_MAX = max(B_S) * CPG * SPC
    assert TROWS_MAX <= 32767, TROWS_MAX

    for c, st in enumerate(cores):
        deg, s0, r, blk_chunks = st["deg"], st["s0"], st["r"], st["blk_chunks"]
        NL = st["NL"]
        # re-map block-local chunks to global chunks with batch alignment
        nb = len(blk_chunks)
        kb_batch = np.zeros(nb, np.int64)
        for s in range(S_BATCH):
            kb_batch[KB0[s]:KB0[s + 1]] = s
        blk_chunk0 = np.zeros(nb + 1, np.int64)
        cur_chunk = 0
        for kb in range(nb):
            s = int(kb_batch[kb])
            if kb == KB0[s]:
                cur_chunk = G0[s] * CPG
            blk_chunk0[kb] = cur_chunk
            cur_chunk += blk_chunks[kb]
            assert cur_chunk <= G0[s + 1] * CPG
        blk_chunk0[nb:] = cur_chunk
        # global slot of each node's run start (s0 was block-sequential)
        # recompute shift: s0 stored with blocks packed back-to-back from 0;
        # block kb originally started at chunk sum(blk_chunks[:kb]) (aligned)
        orig_start = np.zeros(nb, np.int64)
        acc = 0
        for kb in range(nb):
            orig_start[kb] = acc
            acc += blk_chunks[kb]
        shift = (blk_chunk0[:nb] - orig_start) * SPC  # slots to add per block
        node_blk = np.arange(NL) // P
        s0g = np.where(s0 >= 0, s0 + shift[node_blk], -1)

        # edge stream positions (logical), then swizzle to partition-major
        # DRAM layout: row_new = g*4096 + p*CPG + c  for logical edge
        # (g*4096 + c*128 + p) — so each SBUF partition reads one contiguous
        # 2048-element range per group (cheap DMA descriptors).
        first_edge = np.concatenate([[0], np.cumsum(deg)])[:-1]
        epos_base = np.repeat(4 * s0g[deg > 0], deg[deg > 0])
        within = np.arange(len(st["lrow"])) - np.repeat(first_edge[deg > 0], deg[deg > 0])
        epos = epos_base + within
        assert epos.max(initial=-1) < E_PAD * 1, (epos.max(initial=-1), E_PAD)
        eg = epos // GROUP_E
        ec = (epos % GROUP_E) // P
        ep = epos % P
        epos = eg * GROUP_E + ep * CPG + ec

        # table rows for hi / lo slots (batch-local)
        g0_arr = np.asarray(G0[:-1])

        def slot_to_row(sl_g):
            ch = sl_g // SPC
            sl = sl_g % SPC
            g = ch // CPG
            batch = np.searchsorted(np.asarray(G0[1:]), g, side="right")
            m = (ch % CPG) // 8
            j = (ch % CPG) % 8
            gg = g - g0_arr[batch]
            return gg * 1024 + (32 * m + sl) * 8 + j

        hi = np.where(s0g >= 0, s0g + r - 1, 0)
        lo = np.where(s0g >= 0, s0g - 1, 0)
        hi_row = np.where(s0g >= 0, slot_to_row(hi), 0).astype(np.int64)
        lo_row = np.where(s0g >= 0, slot_to_row(lo), 0).astype(np.int64)
        # pad nodes to NC_NODES
        hi_row = np.pad(hi_row, (0, NC_NODES - NL))
        lo_row = np.pad(lo_row, (0, NC_NODES - NL))
        assert hi_row.max() < TROWS_MAX and lo_row.max() < TROWS_MAX

        st.update(epos=epos, hi_row=hi_row, lo_row=lo_row)
    cfg = dict(B_S=B_S, G0=G0, NGROUPS=NGROUPS, E_PAD=E_PAD)
    return cores, cfg


def _wrap_idx(a, npart_rep=8):
    """[M] -> [128, M//16] int16, F-wrapped 16-row block replicated 8x."""
    m = a.reshape(-1, 16).T.astype(np.int16)          # [16, M/16]
    return np.tile(m, (npart_rep, 1))


def _fold_weights(node_w, node_b, edge_w, edge_b, query, in_w, in_b, out_w, out_b):
    D = query.shape[-1]
    dh = D // NUM_HEADS
    wq, wk, wv = in_w[:D], in_w[D:2 * D], in_w[2 * D:]
    bq, bk, bv = in_b[:D], in_b[D:2 * D], in_b[2 * D:]
    q = (query[0] @ wq.T + bq).reshape(NUM_HEADS, dh)
    s_w = np.einsum("hj,hjd->dh", q, wk.reshape(NUM_HEADS, dh, D)) / np.sqrt(dh)
    s_b = np.einsum("hj,hj->h", q, bk.reshape(NUM_HEADS, dh)) / np.sqrt(dh)
    # x_aug = [h(128) | meanA(64) | has_edge | 1] -> h_proj mapping A1 [194, 256]
    A1 = np.concatenate([node_w.T, edge_w.T, edge_b[None, :], node_b[None, :]], axis=0)
    M2 = np.concatenate([wv.T, s_w], axis=1)          # [256, 260]
    Wbig = A1 @ M2                                     # [194, 260]
    Wbig[-1, :256] += bv
    Wbig[-1, 256:] += s_b
    return Wbig.astype(np.float32)


# ------------------------------------------------------- numpy device model
def _numpy_device_model(cores, cfg, h, edge_attr, batch, Wbig, out_w, out_b):
    """Bit-approximate emulation of the device program (bf16 rounding where
    the device rounds) — used to validate packing/indexing host-side."""
    import ml_dtypes
    bf = lambda x: x.astype(ml_dtypes.bfloat16).astype(np.float32)
    B_S, G0, E_PAD = cfg["B_S"], cfg["G0"], cfg["E_PAD"]
    outs = []
    for c, st in enumerate(cores):
        NL = st["NL"]
        stream = np.zeros((E_PAD, 64), np.float32)
        stream[st["epos"]] = edge_attr[st["eord"]]
        streamb = bf(stream)
        # P4 tables per batch
        tables = []
        for s in range(S_BATCH):
            tab = np.zeros((32767, 64), np.float32)
            for gg in range(B_S[s]):
                g = G0[s] + gg
                for ch in range(CPG):
                    cdat = streamb[g * GROUP_E + np.arange(P) * CPG + ch]
                    pre = np.add.reduceat(cdat, np.arange(0, P, SLOT), 0).cumsum(0)
                    m, j = ch // 8, ch % 8
                    sl = np.arange(SPC)
                    tab[gg * 1024 + (32 * m + sl) * 8 + j] = pre
            tables.append(tab)
        nodechunk_batch = np.searchsorted(np.asarray(KB0[1:]),
                                          np.arange(NC_NODES) // P, side="right")
        tab_all = np.stack(tables)
        phi = tab_all[nodechunk_batch, st["hi_row"]]
        plo = tab_all[nodechunk_batch, st["lo_row"]]
        deg = np.pad(st["deg"], (0, NC_NODES - NL)).astype(np.float32)
        inv = 1.0 / np.maximum(deg, 1.0)
        meanA = bf((phi - plo) * inv[:, None])
        hase = (deg > 0).astype(np.float32)
        hpad = np.zeros((NC_NODES, 128), np.float32)
        hpad[:NL] = h[st["n0"]:st["n1"]]
        xaug = np.concatenate([bf(hpad), meanA, bf(hase[:, None]),
                               np.ones((NC_NODES, 1), np.float32)], 1)
        vs = bf(xaug) @ bf(Wbig)
        v, sc = vs[:, :256], vs[:, 256:]
        w = np.exp(sc)
        bl = np.full(NC_NODES, -1, np.int64)
        bl[:NL] = batch[st["n0"]:st["n1"]] - GL * c
        onehot = (bl[:, None] == np.arange(GL)[None, :]).astype(np.float32)
        wv4 = np.concatenate([bf(w[:, :, None] * v.reshape(-1, 4, 64)).reshape(-1, 256),
                              bf(w)], 1)
        U = bf(onehot).T @ wv4
        den = np.maximum(U[:, 256:], 1e-30)
        pooled = U[:, :256].reshape(GL, 4, 64) / den[:, :, None]
        o = pooled.reshape(GL, 256) @ out_w.T + out_b
        outs.append(o)
    return np.concatenate(outs).reshape(G_TOTAL, 1, 256)


# ------------------------------------------------------------- bass program
def _build_program(cfg):
    import concourse.bacc as bacc
    import concourse.mybir as mybir
    import concourse.tile as tile

    F32 = mybir.dt.float32
    BF16 = mybir.dt.bfloat16
    I16 = mybir.dt.int16
    AF = mybir.ActivationFunctionType
    B_S, G0, NGROUPS, E_PAD = cfg["B_S"], cfg["G0"], cfg["NGROUPS"], cfg["E_PAD"]
    NKB = NC_NODES // P            # 52 node chunks

    nc = bacc.Bacc("TRN2", num_devices=CORES, num_swdge_queues=4)
    es_d = nc.dram_tensor("es", [E_PAD, 64], BF16, kind="ExternalInput")
    h_d = nc.dram_tensor("h", [P, NC_NODES // P, 128], BF16, kind="ExternalInput")
    meta_d = nc.dram_tensor("meta", [P, NC_NODES // P, 4], F32, kind="ExternalInput")
    hi_d = nc.dram_tensor("hi", [P, NC_NODES // 16], I16, kind="ExternalInput")
    lo_d = nc.dram_tensor("lo", [P, NC_NODES // 16], I16, kind="ExternalInput")
    tri_d = nc.dram_tensor("tri", [P, SPC], BF16, kind="ExternalInput")
    idtb_d = nc.dram_tensor("idtb", [P, P], mybir.dt.bfloat16, kind="ExternalInput")
    idtf_d = nc.dram_tensor("idtf", [P, P], F32, kind="ExternalInput")
    iota_d = nc.dram_tensor("iota", [P, GL], F32, kind="ExternalInput")
    wb1_d = nc.dram_tensor("wb1", [128, 260], mybir.dt.bfloat16, kind="ExternalInput")
    wb2_d = nc.dram_tensor("wb2", [66, 260], mybir.dt.bfloat16, kind="ExternalInput")
    owt_d = nc.dram_tensor("owt", [256, 256], F32, kind="ExternalInput")
    ob_d = nc.dram_tensor("ob", [GL, 256], F32, kind="ExternalInput")
    y_d = nc.dram_tensor("y", [GL, 256], F32, kind="ExternalOutput")
    p4t = [nc.dram_tensor(f"p4t{s}", [B_S[s] * 1024, 64], F32, kind="Internal")
           for s in range(S_BATCH)]

    with tile.TileContext(nc) as tc:
        with tc.tile_pool(name="const", bufs=1) as cp, \
             tc.tile_pool(name="sb", bufs=3) as sb, \
             tc.tile_pool(name="big", bufs=1) as bigp, \
             tc.tile_pool(name="ps", bufs=2, space="PSUM") as ps, \
             tc.tile_pool(name="pacc", bufs=1, space="PSUM") as pacc:

            trib = cp.tile([P, SPC], BF16, name="trib")
            nc.sync.dma_start(out=trib[:], in_=tri_d.ap()[:, :])
            idtb = cp.tile([P, P], BF16, name="idtb")
            nc.sync.dma_start(out=idtb[:], in_=idtb_d.ap()[:, :])
            idtf = cp.tile([P, P], F32, name="idtf")
            nc.sync.dma_start(out=idtf[:], in_=idtf_d.ap()[:, :])
            iot = cp.tile([P, GL], F32, name="iot")
            nc.sync.dma_start(out=iot[:], in_=iota_d.ap()[:, :])
            wb1 = cp.tile([128, 260], BF16, name="wb1")
            nc.sync.dma_start(out=wb1[:], in_=wb1_d.ap()[:, :])
            wb2 = cp.tile([66, 260], BF16, name="wb2")
            nc.sync.dma_start(out=wb2[:], in_=wb2_d.ap()[:, :])
            owt = cp.tile([P, 2, 256], F32, name="owt")
            nc.sync.dma_start(out=owt[:], in_=owt_d.ap()[:, :].rearrange("(i p) f -> p i f", p=P))
            obt = cp.tile([GL, 256], F32, name="obt")
            nc.sync.dma_start(out=obt[:], in_=ob_d.ap()[:, :])
            hi_t = cp.tile([P, NC_NODES // 16], I16, name="hi_t")
            nc.sync.dma_start(out=hi_t[:], in_=hi_d.ap()[:, :])
            lo_t = cp.tile([P, NC_NODES // 16], I16, name="lo_t")
            nc.sync.dma_start(out=lo_t[:], in_=lo_d.ap()[:, :])

            # h shipped pre-transposed: partition = feature, free = (chunk, node)
            hsb = bigp.tile([P, NKB, 128], BF16, name="hsb")
            msb = bigp.tile([P, NKB, 4], F32, name="msb")
            phi = bigp.tile([P, NKB, 64], F32, name="phi")
            plo = bigp.tile([P, NKB, 64], F32, name="plo")
            augTall = bigp.tile([P, NKB, P], BF16, name="augTall")
            memall = bigp.tile([P, NKB, GL], BF16, name="memall")

            pool_ps = pacc.tile([GL, 260], F32, name="pool_ps")

            def emit_batch_dense(s):
                k0, KBB = KB0[s], KB_LIST[s]
                am = sb.tile([P, KBB, 64], F32, name="am", tag="am", bufs=2)
                nc.vector.tensor_sub(out=am[:], in0=phi[:, k0:k0 + KBB, :],
                                     in1=plo[:, k0:k0 + KBB, :])
                aug = sb.tile([P, KBB, 66], BF16, name="aug", tag="aug", bufs=2)
                nc.vector.tensor_tensor(
                    out=aug[:, :, :64], in0=am[:],
                    in1=msb[:, k0:k0 + KBB, 0].broadcast_to([P, KBB, 64]),
                    op=mybir.AluOpType.mult)
                nc.vector.tensor_copy(out=aug[:, :, 64:66],
                                      in_=msb[:, k0:k0 + KBB, 1:3])
                nc.vector.tensor_tensor(
                    out=memall[:, k0:k0 + KBB, :],
                    in0=iot[:].broadcast_to([P, GL, KBB]).rearrange("p g k -> p k g"),
                    in1=msb[:, k0:k0 + KBB, 3].broadcast_to([P, KBB, GL]),
                    op=mybir.AluOpType.is_equal)
                for k in range(k0, k0 + KBB):
                    pta = ps.tile([66, P], BF16, name="pta", tag="pta", bufs=2)
                    nc.tensor.transpose(out=pta[:], in_=aug[:, k - k0, :66],
                                        identity=idtb[:])
                    nc.scalar.copy(out=augTall[:66, k, :], in_=pta[:])
                for k in range(k0, k0 + KBB):
                    vs = ps.tile([P, 260], F32, name="vs", tag="vs", bufs=3)
                    nc.tensor.matmul(out=vs[:], lhsT=hsb[:, k, :], rhs=wb1[:],
                                     start=True, stop=False)
                    nc.tensor.matmul(out=vs[:], lhsT=augTall[:66, k, :], rhs=wb2[:],
                                     start=False, stop=True)
                    wsb = sb.tile([P, 4], F32, name="wsb", tag="wsb", bufs=4)
                    nc.scalar.activation(out=wsb[:], in_=vs[:, 256:260], func=AF.Exp)
                    pr = sb.tile([P, 260], BF16, name="pr", tag="pr", bufs=4)
                    nc.vector.tensor_tensor(
                        out=pr[:, :256].rearrange("p (h f) -> p h f", h=NUM_HEADS),
                        in0=vs[:, :256].rearrange("p (h f) -> p h f", h=NUM_HEADS),
                        in1=wsb[:].broadcast_to([P, NUM_HEADS, 64]),
                        op=mybir.AluOpType.mult)
                    nc.vector.tensor_copy(out=pr[:, 256:260], in_=wsb[:])
                    nc.tensor.matmul(out=pool_ps[:], lhsT=memall[:, k, :], rhs=pr[:],
                                     start=(k == 0), stop=(k == NKB - 1))

            for g in range(NGROUPS):
                et = sb.tile([P, CPG, 64], BF16, name="et", tag="et", bufs=6)
                nc.sync.dma_start(
                    out=et[:],
                    in_=es_d.ap()[g * GROUP_E:(g + 1) * GROUP_E, :]
                        .rearrange("(p c) f -> p c f", p=P))
                pp = ps.tile([P, 512], F32, name="pp", tag="pp", bufs=2)
                for m in range(4):
                    nc.tensor.matmul(
                        out=pp[32 * m:32 * m + 32, :],
                        lhsT=trib[:],
                        rhs=et[:, 8 * m:8 * m + 8, :].rearrange("p c f -> p (c f)"),
                        start=True, stop=True,
                        tile_position=(0, 32 * m))
                p4sb = sb.tile([P, 512], F32, name="p4sb", tag="p4sb", bufs=6)
                nc.vector.tensor_copy(out=p4sb[:], in_=pp[:])
                import bisect
                s = bisect.bisect_right(G0, g) - 1
                gg = g - G0[s]
                # spill on the ACT HWDGE queue so edge loads (sync queue) flow
                nc.scalar.dma_start(
                    out=p4t[s].ap()[gg * 1024:(gg + 1) * 1024, :]
                        .rearrange("(q x) f -> q (x f)", q=P),
                    in_=p4sb[:])
                if gg != B_S[s] - 1:
                    continue
                if s == 0:
                    # big node loads on the scalar HWDGE queue: off the
                    # latency-critical sync queue that feeds edge tiles
                    nc.gpsimd.dma_start(out=hsb[:], in_=h_d.ap()[:, :, :])
                    nc.gpsimd.dma_start(out=msb[:], in_=meta_d.ap()[:, :, :])
                # ---- batch s fully spilled: issue gathers now; defer the
                # dense work one batch so gather latency hides behind the
                # next batch's prefix matmuls (PE queue is in-order).
                k0, KBB = KB0[s], KB_LIST[s]
                c0 = KB0[s] * 8
                halves = [(0, KBB // 2), (KBB // 2, KBB)]
                qn = 0
                for tgt, idxt_t in ((phi, hi_t), (plo, lo_t)):
                    for (a, b) in halves:
                        nidx = (b - a) * P
                        nc.gpsimd.dma_gather(
                            out_ap=tgt[:, k0 + a:k0 + b, :],
                            in_ap=p4t[s].ap()[:, :],
                            idxs_ap=idxt_t[:, c0 + a * 8:c0 + b * 8],
                            num_idxs=nidx, num_idxs_reg=nidx, elem_size=64,
                            single_packet=False, queue_num=qn)
                        qn = (qn + 1) % 4
                if s >= DENSE_LAG:
                    emit_batch_dense(s - DENSE_LAG)

            for s in range(max(0, S_BATCH - DENSE_LAG), S_BATCH):
                emit_batch_dense(s)

            # ---- final: normalize + output projection
            den = sb.tile([GL, 4], F32, name="den")
            nc.vector.tensor_scalar_max(out=den[:], in0=pool_ps[:, 256:260],
                                        scalar1=1e-30)
            rden = sb.tile([GL, 4], F32, name="rden")
            nc.vector.reciprocal(out=rden[:], in_=den[:])
            pn = sb.tile([GL, 256], F32, name="pn")
            for hh in range(NUM_HEADS):
                nc.vector.tensor_scalar_mul(out=pn[:, 64 * hh:64 * hh + 64],
                                            in0=pool_ps[:, 64 * hh:64 * hh + 64],
                                            scalar1=rden[:, hh:hh + 1])
            pnT = sb.tile([P, 2, GL], F32, name="pnT")
            for i in range(2):
                ptp = ps.tile([P, GL], F32, name="ptp", tag="pta", bufs=2)
                nc.tensor.transpose(out=ptp[:], in_=pn[:, i * P:(i + 1) * P],
                                    identity=idtf[:GL, :GL])
                nc.vector.tensor_copy(out=pnT[:, i, :], in_=ptp[:])
            ops_t = ps.tile([GL, 256], F32, name="ops_t", tag="pta", bufs=2)
            for i in range(2):
                nc.tensor.matmul(out=ops_t[:], lhsT=pnT[:, i, :], rhs=owt[:, i, :],
                                 start=(i == 0), stop=(i == 1))
            osb = sb.tile([GL, 256], F32, name="osb")
            nc.vector.tensor_add(out=osb[:], in0=ops_t[:], in1=obt[:])
            nc.sync.dma_start(out=y_d.ap()[:, :], in_=osb[:])

    nc.finalize()
    return nc


_CACHE = {}


def _get_program(cfg):
    key = tuple(cfg["B_S"])
    if key not in _CACHE:
        _CACHE[key] = _build_program(cfg)
    return _CACHE[key]


def kernel(h, edge_index, edge_attr, batch, num_graphs,
           node_w, node_b, edge_w, edge_b, query, in_w, in_b, out_w, out_b,
           _trace=False, _numpy_only=False):
    import ml_dtypes
    h = np.asarray(h, np.float32)
    edge_attr = np.asarray(edge_attr, np.float32)
    batch_np = np.asarray(batch, np.int64)
    assert int(num_graphs) == G_TOTAL

    cores, cfg = _prep(h, edge_index, edge_attr, batch_np)
    Wbig = _fold_weights(np.asarray(node_w, np.float32), np.asarray(node_b, np.float32),
                         np.asarray(edge_w, np.float32), np.asarray(edge_b, np.float32),
                         np.asarray(query, np.float32), np.asarray(in_w, np.float32),
                         np.asarray(in_b, np.float32), np.asarray(out_w, np.float32),
                         np.asarray(out_b, np.float32))
    if _numpy_only:
        return _numpy_device_model(cores, cfg, h, edge_attr, batch_np, Wbig,
                                   np.asarray(out_w, np.float32),
                                   np.asarray(out_b, np.float32))

    bf16 = ml_dtypes.bfloat16
    tri = (np.arange(P)[:, None] // SLOT <= np.arange(SPC)[None, :]).astype(np.float32)
    idt = np.eye(P)
    iota = np.tile(np.arange(GL, dtype=np.float32)[None, :], (P, 1))
    shared = dict(
        tri=tri.astype(bf16),
        idtb=idt.astype(bf16), idtf=idt.astype(np.float32),
        iota=iota,
        wb1=Wbig[:128].astype(bf16), wb2=Wbig[128:].astype(bf16),
        owt=np.ascontiguousarray(np.asarray(out_w, np.float32).T),
        ob=np.tile(np.asarray(out_b, np.float32)[None, :], (GL, 1)),
    )
    in_maps = []
    for c, st in enumerate(cores):
        NL = st["NL"]
        stream = np.zeros((cfg["E_PAD"], 64), np.float32)
        stream[st["epos"]] = edge_attr[st["eord"]]
        hpad = np.zeros((NC_NODES, 128), np.float32)
        hpad[:NL] = h[st["n0"]:st["n1"]]
        deg = np.pad(st["deg"], (0, NC_NODES - NL)).astype(np.float32)
        meta = np.stack([1.0 / np.maximum(deg, 1.0),
                         (deg > 0).astype(np.float32),
                         np.ones(NC_NODES, np.float32),
                         np.concatenate([batch_np[st["n0"]:st["n1"]] - GL * c,
                                         np.full(NC_NODES - NL, -1.0)]).astype(np.float32)],
                        axis=1).astype(np.float32)
        h3 = np.ascontiguousarray(hpad.reshape(-1, P, 128).transpose(2, 0, 1))
        meta3 = np.ascontiguousarray(meta.reshape(-1, P, 4).transpose(1, 0, 2))
        in_maps.append(dict(es=stream.astype(bf16), h=h3.astype(bf16), meta=meta3,
                            hi=_wrap_idx(st["hi_row"]), lo=_wrap_idx(st["lo_row"]),
                            **shared))

    from concourse.bass_utils import run_bass_kernel_spmd
    nc = _get_program(cfg)
    res = run_bass_kernel_spmd(nc, in_maps, core_ids=list(range(CORES)),
                               trace=_trace)
    out = np.concatenate([np.asarray(res.results[c]["y"], np.float32)
                          for c in range(CORES)])
    kernel._last_result = res
    return out.reshape(G_TOTAL, 1, 256)
